# revision 39
# baseline (speedup 1.0000x reference)
"""Multi-head attention forward on 8 Trainium2 NeuronCores.

Problem (hardcoded): B=4, N=M=2048, D=1024, H=16, HS=64, OUT=1024, fp32.

Sharding: 8 cores = 4 batches x 2 head-groups of 8 heads. Each core
computes a partial output [2048, 1024] = sum over its 8 heads of
softmax((X_q Wq_h)(X_k Wk_h)^T / 8) (X_v Wv_h) Wo_h.  Host sums the two
head-group partials per batch and adds the projection bias.

v2 design notes (vs the 749us baseline):
  * bf16 dataflow everywhere the PE touches data (1 cyc/row, incl.
    transposes which were fp32 = 2 cyc/row); f32 only in PSUM
    accumulators and the softmax denominators/reciprocals. Total rel
    err ~3e-3 vs the 2e-2 gate.
  * Phase A is a software pipeline: DMA -> ACT cast to bf16 ->
    PE transpose -> project, with the q projection interleaved into
    xk's transposes (and k-proj into xv's) so the PE instruction
    stream is dense from the first tile on. This keeps the HAM
    activity monitor promoting the PE clock to 2.4 GHz without the
    baseline's throwaway warm-up bursts (which burned 41us of PE).
  * Attention loops nh (n-half) outer / head inner. exp runs on the
    ACT engine which is the true bottleneck (33.5M exps/core @ 1.2GHz
    ~ 263us); lg PSUM pool depth 3 lets the PE run up to 3 logit
    tiles ahead so ACT never waits. Normalization + output projection
    of half 0 are dripped into half 1's ACT-bound slack.
  * Odd heads' ctx rows move to partitions 64:128 via SBUF->SBUF
    partition-hop DMA instead of PE shift matmuls.
"""

import os
import sys

import numpy as np

for _p in ("/opt/trn_rl_repo",):
    if _p not in sys.path and os.path.isdir(_p):
        sys.path.insert(0, _p)

B, N, M, D = 4, 2048, 2048, 1024
H, HS, OUT = 16, 64, 1024
HL = 8          # heads per core
P = 128
NPAIR = HL // 2  # head pairs per core
DT = D // P      # 8 d-tiles
NT = N // P      # 16 n-tiles
MT = M // P      # 16 m-tiles
NH = 1024       # n-half width for the attention loop
PIPE = 4        # ctx trails logits/exp by this many m-tiles


def build_mha(tc, ins, out_ap):
    import concourse.bass as bass
    from concourse import mybir

    nc = tc.nc
    f32 = mybir.dt.float32
    bf16 = mybir.dt.bfloat16
    f32r = mybir.dt.float32r

    def r(ap):
        return ap.bitcast(f32r)

    xq, xk, xv = ins["xq"], ins["xk"], ins["xv"]
    wq, wk, wv, wo = ins["wq"], ins["wk"], ins["wv"], ins["wo"]

    import contextlib

    with contextlib.ExitStack() as ctx:
        # ---- constant tiles ----
        const = ctx.enter_context(tc.tile_pool(name="const", bufs=1))
        identity = const.tile([P, P], f32)
        from concourse.masks import make_identity
        make_identity(nc, identity)
        identity_bf = const.tile([P, P], bf16)
        nc.gpsimd.tensor_copy(identity_bf[:], identity[:])


        # ---- persistent activations ----
        act_pool = ctx.enter_context(tc.tile_pool(name="acts", bufs=1))
        # QT/KT: one [128, 2048] bf16 tile per head pair; partitions 0:64
        # head 2p, 64:128 head 2p+1.
        qt = [act_pool.tile([P, N], bf16, name=f"qt{p}", tag=f"qt{p}") for p in range(NPAIR)]
        kt = [act_pool.tile([P, M], bf16, name=f"kt{p}", tag=f"kt{p}") for p in range(NPAIR)]
        # V: per m-tile [128, 8 heads, 65]; col 64 is ones (softmax denom).
        v_all = [act_pool.tile([P, HL, 65], bf16, name=f"v{t}", tag=f"v{t}") for t in range(MT)]
        # normalized ctx rows, pair-stacked (even head 0:64, odd 64:128)
        ctxn = [act_pool.tile([P, N], bf16, name=f"ctxn{p}", tag=f"ctxn{p}")
                for p in range(NPAIR)]
        # wo [8, 64, 1024] -> SBUF [128(s*64+o), pair, 1024] bf16
        wo_sb = act_pool.tile([P, NPAIR, OUT], bf16, name="wo_sb", tag="wo_sb")

        # ---- phase A: load, cast, transpose, project (pipelined) ----
        # Chain order q -> v -> k; each projection interleaves with the next
        # tensor's transposes so the PE stream is dense. The k projection is
        # deferred into the attention scope (pair 0 first, rest dripped into
        # the first heads' ACT-bound slack).
        xt_pool = ctx.enter_context(tc.tile_pool(name="xt", bufs=2))
        wk_pool = ctx.enter_context(tc.tile_pool(name="wk_pool", bufs=1))
        with tc.tile_pool(name="x_stream", bufs=4) as x_stream, \
             tc.tile_pool(name="xb_pool", bufs=4) as xb_pool, \
             tc.tile_pool(name="w_pool", bufs=2) as w_pool, \
             tc.tile_pool(name="w_stream", bufs=2) as w_stream, \
             tc.tile_pool(name="tp_psum", bufs=4, space="PSUM") as tp_psum, \
             tc.tile_pool(name="proj_psum", bufs=2, space="PSUM") as proj_psum:

            def load_w(w_dram, nm, pool=None):
                # w [8, 1024, 64] -> SBUF [128(d in tile), dt, h, 64] bf16
                w_sb = (pool or w_pool).tile([P, DT, HL, HS], bf16, name=nm, tag="w_sb")
                for dt_i in range(DT):
                    st = w_stream.tile([P, HL, HS], f32, name="wst", tag="wst")
                    nc.sync.dma_start(
                        st[:],
                        w_dram[:, dt_i * P:(dt_i + 1) * P, :].rearrange("h p o -> p h o"))
                    nc.scalar.copy(w_sb[:, dt_i, :, :], st[:])
                return w_sb

            def transpose_tile(x_dram, xt_tile, t):
                # one [128, 1024] row-tile: DMA, ACT-cast to bf16, 8 PE
                # transposes into 2 psum groups, evict to xt (DVE/Pool).
                x_t = x_stream.tile([P, D], f32, name="x_t", tag="x_t")
                nc.sync.dma_start(x_t[:], x_dram[t * P:(t + 1) * P, :])
                xb = xb_pool.tile([P, D], bf16, name="xb", tag="xb")
                nc.scalar.copy(xb[:], x_t[:])
                for g in range(2):
                    tp = tp_psum.tile([P, 4 * P], bf16, name="tp", tag="tp")
                    for i in range(4):
                        dt_i = 4 * g + i
                        nc.tensor.transpose(
                            tp[:, i * P:(i + 1) * P],
                            xb[:, dt_i * P:(dt_i + 1) * P],
                            identity_bf[:])
                    nc.vector.tensor_copy(
                        xt_tile[:, 4 * g:4 * g + 4, t * P:(t + 1) * P],
                        tp[:].rearrange("p (d n) -> p d n", d=4))

            def qk_proj_chunk(w_sb, xt_tile, dst, p, c):
                ps = proj_psum.tile([P, 512], f32, name="qk_ps", tag="qk_ps")
                for dt_i in range(DT):
                    nc.tensor.matmul(
                        ps[:],
                        w_sb[:, dt_i, 2 * p:2 * p + 2, :],
                        xt_tile[:, dt_i, c * 512:(c + 1) * 512],
                        start=(dt_i == 0), stop=(dt_i == DT - 1),
                    )
                nc.vector.tensor_copy(dst[p][:, c * 512:(c + 1) * 512], ps[:])

            chunks = [(p, c) for p in range(NPAIR) for c in range(N // 512)]

            # q chain: DMA+cast+transpose all 16 tiles (DMA-paced)
            wq_sb = load_w(wq, "wq_sb")
            xqt = xt_pool.tile([P, DT, N], bf16, name="xqt", tag="xT")
            for t in range(NT):
                transpose_tile(xq, xqt, t)
            # v transposes interleaved with q projection (PE stays dense)
            wv_sb = load_w(wv, "wv_sb")
            xvt = xt_pool.tile([P, DT, M], bf16, name="xvt", tag="xT")
            for i in range(NT):
                qk_proj_chunk(wq_sb, xqt, qt, *chunks[i])
                transpose_tile(xv, xvt, i)
            # k transposes interleaved with the v projection
            wk_sb = load_w(wk, "wk_sb", pool=wk_pool)
            xkt = xt_pool.tile([P, DT, M], bf16, name="xkt", tag="xT")
            for t in range(MT):
                # V projection: v_all[t][:, h, 0:64] = (x_v @ Wv_h)[m-tile t]
                ps = proj_psum.tile([P, 512], f32, name="qk_ps", tag="qk_ps")
                for dt_i in range(DT):
                    nc.tensor.matmul(
                        ps[:],
                        xvt[:, dt_i, t * P:(t + 1) * P],
                        wv_sb[:, dt_i, :, :],
                        start=(dt_i == 0), stop=(dt_i == DT - 1),
                    )
                nc.vector.tensor_copy(
                    v_all[t][:, :, 0:64], ps[:].rearrange("p (h o) -> p h o", h=HL))
                nc.vector.memset(v_all[t][:, :, 64:65], 1.0)
                transpose_tile(xk, xkt, t)


        # ---- phase B: k-proj tail + attention + norm + output projection ----
        with tc.tile_pool(name="et", bufs=7) as et_pool, \
             tc.tile_pool(name="stg", bufs=2) as stg_pool, \
             tc.tile_pool(name="rec", bufs=2) as rec_pool, \
             tc.tile_pool(name="wo_stream", bufs=2) as wo_stream, \
             tc.tile_pool(name="ot", bufs=2) as ot_pool, \
             tc.tile_pool(name="lg_psum", bufs=3, space="PSUM") as lg_psum, \
             tc.tile_pool(name="ctx_psum", bufs=1, space="PSUM") as ctx_psum:

            # wo load + cast (DVE; ACT is the attention bottleneck)
            for s in range(2):
                for c4 in range(4):
                    wst = wo_stream.tile([64, NPAIR, 256], f32, name="wo_st", tag="wo_st")
                    nc.sync.dma_start(
                        wst[:],
                        wo[s::2, :, c4 * 256:(c4 + 1) * 256].rearrange("pp o d -> o pp d"))
                    nc.vector.tensor_copy(
                        wo_sb[s * 64:(s + 1) * 64, :, c4 * 256:(c4 + 1) * 256], wst[:])

            # PE filler queue: thunks emitted at fixed m-tile slots inside
            # attention heads, filling the PE while ACT (exp) is the pacer.
            fillers = []

            def kproj_chunk(p, c):
                ps = lg_psum.tile([P, 512], f32, name="k_ps", tag="lg")
                for dt_i in range(DT):
                    nc.tensor.matmul(
                        ps[:],
                        wk_sb[:, dt_i, 2 * p:2 * p + 2, :],
                        xkt[:, dt_i, c * 512:(c + 1) * 512],
                        start=(dt_i == 0), stop=(dt_i == DT - 1),
                    )
                nc.vector.tensor_copy(kt[p][:, c * 512:(c + 1) * 512], ps[:])

            def outproj_tile(t):
                # out rows [t*128:(t+1)*128] = sum_p ctxn[p]^T @ wo[p]
                ot = ot_pool.tile([P, OUT], f32, name="ot", tag="ot")
                for c in range(OUT // 512):
                    ops = lg_psum.tile([P, 512], f32, name="ops", tag="lg")
                    for p in range(NPAIR):
                        nc.tensor.matmul(
                            ops[:],
                            ctxn[p][:, t * P:(t + 1) * P],
                            wo_sb[:, p, c * 512:(c + 1) * 512],
                            start=(p == 0), stop=(p == NPAIR - 1),
                        )
                    nc.vector.tensor_copy(ot[:, c * 512:(c + 1) * 512], ops[:])
                nc.sync.dma_start(out_ap[t * P:(t + 1) * P, :], ot[:])

            def finalize_head(cps, nh, hl):
                # normalize-evict, fully off the PE: 1/denominator (DVE,
                # from PSUM), bf16 cast (DVE), row broadcast (Pool), multiply
                # into the staged ctx rows (DVE), DMA-hop into ctxn.
                n0 = nh * NH
                p_i, s = divmod(hl, 2)
                den = rec_pool.tile([1, NH], f32, name="den", tag="den")
                nc.vector.tensor_copy(den[:], cps[64:65, :])
                rec_f = rec_pool.tile([1, NH], f32, name="rec_f", tag="rec_f")
                nc.vector.reciprocal_approx_fast(rec_f[:], den[:])
                rec_b = rec_pool.tile([1, NH], bf16, name="rec_b", tag="rec_b")
                nc.vector.tensor_copy(rec_b[:], rec_f[:])
                bcst = stg_pool.tile([64, NH], bf16, name="bcst", tag="bcst")
                nc.gpsimd.partition_broadcast(bcst[:], rec_b[:])
                stg = stg_pool.tile([64, NH], bf16, name="stg", tag="stg")
                nc.vector.tensor_mul(stg[:], cps[0:64, :], bcst[:])
                nc.sync.dma_start(
                    ctxn[p_i][s * 64:(s + 1) * 64, n0:n0 + NH], stg[:])

            def attention_head(nh, hl):
                n0 = nh * NH
                p_i, s = divmod(hl, 2)
                pr = slice(s * 64, s * 64 + 64)   # partition range of this head
                cps = ctx_psum.tile([P, NH], f32, name="cps", tag="cps")
                ets = {}

                def emit_logits(t):
                    et = et_pool.tile([P, NH], bf16, name="et", tag="et")
                    ets[t] = et
                    lg = lg_psum.tile([P, NH], f32, name="lg", tag="lg")
                    for c in range(NH // 512):
                        nc.tensor.matmul(
                            lg[:, c * 512:(c + 1) * 512],
                            kt[p_i][pr, t * P:(t + 1) * P],
                            qt[p_i][pr, n0 + c * 512:n0 + (c + 1) * 512],
                            start=True, stop=True,
                        )
                    nc.scalar.activation(
                        et[:], lg[:], mybir.ActivationFunctionType.Exp, scale=0.125)

                def emit_ctx(t):
                    et = ets.pop(t)
                    for c in range(NH // 512):
                        nc.tensor.matmul(
                            cps[0:65, c * 512:(c + 1) * 512],
                            v_all[t][:, hl, :],
                            et[:, c * 512:(c + 1) * 512],
                            start=(t == 0), stop=(t == MT - 1),
                        )

                for t in range(MT):
                    emit_logits(t)
                    if t >= PIPE:
                        emit_ctx(t - PIPE)
                    if t in (5, 11) and fillers:
                        fillers.pop(0)()
                for t in range(MT - PIPE, MT):
                    emit_ctx(t)
                finalize_head(cps, nh, hl)

            # k projection: pair 0 up front, pairs 1-3 dripped into the first
            # heads (pair p's kt is ready just before head 2p runs).
            for c in range(N // 512):
                kproj_chunk(0, c)
            fillers.extend(
                (lambda p=p, c=c: kproj_chunk(p, c)) for (p, c) in chunks[4:])

            for nh in range(N // NH):
                for hl in range(HL):
                    attention_head(nh, hl)
                    if nh == 1 and hl == 0:
                        # half 0 fully evicted (after (1,0)'s finalize);
                        # drip its out-projection into half 1's ACT slack.
                        fillers.extend(
                            (lambda t=t: outproj_tile(t)) for t in range(6))
            # tail: reserved out-proj tiles cover the last head's finalize
            # latency so the PE clock stays promoted
            outproj_tile(6)
            outproj_tile(7)
            for t in range(NT // 2, NT):
                outproj_tile(t)


def build_nc():
    import concourse.bacc as bacc
    import concourse.tile as tile
    from concourse import mybir

    nc = bacc.Bacc("TRN2", target_bir_lowering=False, debug=False)
    f32 = mybir.dt.float32
    ins = {
        "xq": nc.dram_tensor("xq", (N, D), f32, kind="ExternalInput").ap(),
        "xk": nc.dram_tensor("xk", (M, D), f32, kind="ExternalInput").ap(),
        "xv": nc.dram_tensor("xv", (M, D), f32, kind="ExternalInput").ap(),
        "wq": nc.dram_tensor("wq", (HL, D, HS), f32, kind="ExternalInput").ap(),
        "wk": nc.dram_tensor("wk", (HL, D, HS), f32, kind="ExternalInput").ap(),
        "wv": nc.dram_tensor("wv", (HL, D, HS), f32, kind="ExternalInput").ap(),
        "wo": nc.dram_tensor("wo", (HL, HS, OUT), f32, kind="ExternalInput").ap(),
    }
    out_ap = nc.dram_tensor("out", (N, OUT), f32, kind="ExternalOutput").ap()
    with tile.TileContext(nc) as tc:
        build_mha(tc, ins, out_ap)
    nc.compile()
    return nc


def make_in_maps(inputs):
    q = np.ascontiguousarray(np.asarray(inputs["query"], dtype=np.float32))
    k = np.ascontiguousarray(np.asarray(inputs["key"], dtype=np.float32))
    v = np.ascontiguousarray(np.asarray(inputs["value"], dtype=np.float32))
    wq = np.asarray(inputs["query_kernel"], dtype=np.float32)
    wk = np.asarray(inputs["key_kernel"], dtype=np.float32)
    wv = np.asarray(inputs["value_kernel"], dtype=np.float32)
    wo = np.asarray(inputs["projection_kernel"], dtype=np.float32)
    in_maps = []
    for c in range(8):
        b, hg = divmod(c, 2)
        hs = slice(hg * HL, (hg + 1) * HL)
        in_maps.append({
            "xq": q[b], "xk": k[b], "xv": v[b],
            "wq": np.ascontiguousarray(wq[hs]),
            "wk": np.ascontiguousarray(wk[hs]),
            "wv": np.ascontiguousarray(wv[hs]),
            "wo": np.ascontiguousarray(wo[hs]),
        })
    return in_maps


def combine(results, bias):
    out = np.empty((B, N, OUT), dtype=np.float32)
    for b in range(B):
        out[b] = results[2 * b]["out"] + results[2 * b + 1]["out"]
    out += np.asarray(bias, dtype=np.float32)[None, None, :]
    return out


_NC_CACHE = None
_LDW_PATCHED = False


def _enable_ldw_opt():
    """No-op (kept for test.py compat). The v1 kernel forced
    --enable-ldw-opt=true to dedupe f32r stationary reloads; with bf16
    stationaries legalization emits standalone InstLdweights which
    walrus rejects under that flag, and the loads pipeline under the
    matmuls anyway."""
    return


def kernel(**inputs):
    global _NC_CACHE
    from concourse import bass_utils
    _enable_ldw_opt()

    if _NC_CACHE is None:
        _NC_CACHE = build_nc()
    nc = _NC_CACHE
    in_maps = make_in_maps(inputs)
    res = bass_utils.run_bass_kernel_spmd(nc, in_maps, core_ids=list(range(8)))
    return combine(res.results, inputs["projection_bias"])


# revision 40
# speedup vs baseline: 1.0060x; 1.0060x over previous
"""Multi-head attention forward on 8 Trainium2 NeuronCores.

Problem (hardcoded): B=4, N=M=2048, D=1024, H=16, HS=64, OUT=1024, fp32.

Sharding: 8 cores = 4 batches x 2 head-groups of 8 heads. Each core
computes a partial output [2048, 1024] = sum over its 8 heads of
softmax((X_q Wq_h)(X_k Wk_h)^T / 8) (X_v Wv_h) Wo_h.  Host sums the two
head-group partials per batch and adds the projection bias.

Design notes (vs the 749us fp32r baseline):
  * bf16 dataflow everywhere the PE touches data (1 cyc/row, incl.
    transposes which were fp32 = 2 cyc/row); f32 only in PSUM
    accumulators and the softmax denominators. Total rel err ~5e-3
    vs the 2e-2 gate.
  * Phase A is a software pipeline: DMA -> ACT cast to bf16 -> PE
    transpose -> project, with the q projection interleaved into xk's
    transposes (and k-proj into xv's) so the PE instruction stream is
    dense from the first tile on. This keeps the HAM activity monitor
    promoting the PE clock to 2.4 GHz without the baseline's throwaway
    warm-up bursts.
  * Attention loops nh (n-half) outer / head inner. exp runs on the
    ACT engine which is the true bottleneck (33.5M exps/core @ 1.2GHz
    ~ 300us); lg PSUM pool depth 3 lets the PE run up to 3 logit tiles
    ahead so ACT never waits. The output projection of half 0 is
    dripped into half 1's ACT-bound slack.
  * Softmax denominators staged to SBUF in f32; the normalization uses
    reciprocal_approx_fast (~1.3us vs 6.5us for InstReciprocal), and
    two reserved out-projection tiles keep the PE busy across the
    final reciprocal so the clock stays promoted through the tail.
"""

import os
import sys

import numpy as np

for _p in ("/opt/trn_rl_repo",):
    if _p not in sys.path and os.path.isdir(_p):
        sys.path.insert(0, _p)

B, N, M, D = 4, 2048, 2048, 1024
H, HS, OUT = 16, 64, 1024
HL = 8          # heads per core
P = 128
NPAIR = HL // 2  # head pairs per core
DT = D // P      # 8 d-tiles
NT = N // P      # 16 n-tiles
MT = M // P      # 16 m-tiles
NH = 1024       # n-half width for the attention loop
PIPE = 3        # ctx trails logits/exp by this many m-tiles


def build_mha(tc, ins, out_ap):
    import concourse.bass as bass
    from concourse import mybir

    nc = tc.nc
    f32 = mybir.dt.float32
    bf16 = mybir.dt.bfloat16

    xq, xk, xv = ins["xq"], ins["xk"], ins["xv"]
    wq, wk, wv, wo = ins["wq"], ins["wk"], ins["wv"], ins["wo"]

    import contextlib

    with contextlib.ExitStack() as ctx:
        # ---- constant tiles ----
        const = ctx.enter_context(tc.tile_pool(name="const", bufs=1))
        identity = const.tile([P, P], f32)
        from concourse.masks import make_identity
        make_identity(nc, identity)
        identity_bf = const.tile([P, P], bf16)
        nc.gpsimd.tensor_copy(identity_bf[:], identity[:])
        # head-select masks: hmask[0:HL, h, :] is 1 on partition h, else 0.
        # K=8 lhsT for broadcasting one head's denominator row to 64 partitions.
        hmask_f32 = const.tile([HL, HL, 64], f32)
        nc.gpsimd.memset(hmask_f32[:], 0.0)
        nc.gpsimd.affine_select(
            out=hmask_f32[:],
            in_=hmask_f32[:],
            compare_op=mybir.AluOpType.not_equal,
            fill=1.0,
            base=0,
            # iota = partition - h : zero exactly where partition == h
            pattern=[[-1, HL], [0, 64]],
            channel_multiplier=1,
        )
        hmask_bf = const.tile([HL, HL, 64], bf16)
        nc.gpsimd.tensor_copy(hmask_bf[:], hmask_f32[:])

        # ---- persistent activations ----
        act_pool = ctx.enter_context(tc.tile_pool(name="acts", bufs=1))
        # QT/KT: one [128, 2048] bf16 tile per head pair; partitions 0:64
        # head 2p, 64:128 head 2p+1.
        qt = [act_pool.tile([P, N], bf16, name=f"qt{p}", tag=f"qt{p}") for p in range(NPAIR)]
        kt = [act_pool.tile([P, M], bf16, name=f"kt{p}", tag=f"kt{p}") for p in range(NPAIR)]
        # V: per m-tile [128, 8 heads, 65]; col 64 is ones (softmax denom).
        v_all = [act_pool.tile([P, HL, 65], bf16, name=f"v{t}", tag=f"v{t}") for t in range(MT)]
        # un-normalized ctx rows, pair-stacked (even head 0:64, odd 64:128)
        ctxn = [act_pool.tile([P, N], bf16, name=f"ctxn{p}", tag=f"ctxn{p}")
                for p in range(NPAIR)]
        # softmax denominators (f32 for reciprocal_approx_fast) and their
        # bf16 reciprocals (broadcast-matmul operand)
        sums_all = act_pool.tile([HL, N], f32, name="sums_all", tag="sums_all")
        sums_rec = act_pool.tile([HL, N], bf16, name="sums_rec", tag="sums_rec")
        # wo [8, 64, 1024] -> SBUF [128(s*64+o), pair, 1024] bf16
        wo_sb = act_pool.tile([P, NPAIR, OUT], bf16, name="wo_sb", tag="wo_sb")

        # ---- phase A: load, cast, transpose, project (pipelined) ----
        with tc.tile_pool(name="xt", bufs=2) as xt_pool, \
             tc.tile_pool(name="x_stream", bufs=4) as x_stream, \
             tc.tile_pool(name="xb_pool", bufs=4) as xb_pool, \
             tc.tile_pool(name="w_pool", bufs=2) as w_pool, \
             tc.tile_pool(name="w_stream", bufs=2) as w_stream, \
             tc.tile_pool(name="tp_psum", bufs=4, space="PSUM") as tp_psum, \
             tc.tile_pool(name="proj_psum", bufs=3, space="PSUM") as proj_psum:

            def load_w(w_dram, nm):
                # w [8, 1024, 64] -> SBUF [128(d in tile), dt, h, 64] bf16
                w_sb = w_pool.tile([P, DT, HL, HS], bf16, name=nm, tag="w_sb")
                for dt_i in range(DT):
                    st = w_stream.tile([P, HL, HS], f32, name="wst", tag="wst")
                    nc.sync.dma_start(
                        st[:],
                        w_dram[:, dt_i * P:(dt_i + 1) * P, :].rearrange("h p o -> p h o"))
                    nc.scalar.copy(w_sb[:, dt_i, :, :], st[:])
                return w_sb

            def transpose_tile(x_dram, xt_tile, t):
                # one [128, 1024] row-tile: DMA, ACT-cast to bf16, 8 PE
                # transposes into 2 psum groups, evict to xt (DVE).
                x_t = x_stream.tile([P, D], f32, name="x_t", tag="x_t")
                nc.sync.dma_start(x_t[:], x_dram[t * P:(t + 1) * P, :])
                xb = xb_pool.tile([P, D], bf16, name="xb", tag="xb")
                nc.scalar.copy(xb[:], x_t[:])
                for g in range(2):
                    tp = tp_psum.tile([P, 4 * P], bf16, name="tp", tag="tp")
                    for i in range(4):
                        dt_i = 4 * g + i
                        nc.tensor.transpose(
                            tp[:, i * P:(i + 1) * P],
                            xb[:, dt_i * P:(dt_i + 1) * P],
                            identity_bf[:])
                    nc.vector.tensor_copy(
                        xt_tile[:, 4 * g:4 * g + 4, t * P:(t + 1) * P],
                        tp[:].rearrange("p (d n) -> p d n", d=4))

            def qk_proj_chunk(w_sb, xt_tile, dst, p, c):
                ps = proj_psum.tile([P, 512], f32, name="qk_ps", tag="qk_ps")
                for dt_i in range(DT):
                    nc.tensor.matmul(
                        ps[:],
                        w_sb[:, dt_i, 2 * p:2 * p + 2, :],
                        xt_tile[:, dt_i, c * 512:(c + 1) * 512],
                        start=(dt_i == 0), stop=(dt_i == DT - 1),
                    )
                nc.vector.tensor_copy(dst[p][:, c * 512:(c + 1) * 512], ps[:])

            chunks = [(p, c) for p in range(NPAIR) for c in range(N // 512)]

            # q chain: DMA+cast+transpose all 16 tiles (DMA-paced)
            wq_sb = load_w(wq, "wq_sb")
            xqt = xt_pool.tile([P, DT, N], bf16, name="xqt", tag="xT")
            for t in range(NT):
                transpose_tile(xq, xqt, t)
            # k transposes interleaved with q projection (PE stays dense)
            wk_sb = load_w(wk, "wk_sb")
            xkt = xt_pool.tile([P, DT, M], bf16, name="xkt", tag="xT")
            for i in range(NT):
                qk_proj_chunk(wq_sb, xqt, qt, *chunks[i])
                transpose_tile(xk, xkt, i)
            # v transposes interleaved with k projection
            wv_sb = load_w(wv, "wv_sb")
            xvt = xt_pool.tile([P, DT, M], bf16, name="xvt", tag="xT")
            for i in range(NT):
                qk_proj_chunk(wk_sb, xkt, kt, *chunks[i])
                transpose_tile(xv, xvt, i)
            # wo load + cast (ACT idle slot), staged in 4KB chunks
            for s in range(2):
                for c4 in range(4):
                    wst = w_stream.tile([64, NPAIR, 256], f32, name="wo_st", tag="wst")
                    nc.sync.dma_start(
                        wst[:],
                        wo[s::2, :, c4 * 256:(c4 + 1) * 256].rearrange("pp o d -> o pp d"))
                    nc.scalar.copy(
                        wo_sb[s * 64:(s + 1) * 64, :, c4 * 256:(c4 + 1) * 256], wst[:])
            # V projection: v_all[t][:, h, 0:64] = (x_v @ Wv_h)[m-tile t]
            for t in range(MT):
                ps = proj_psum.tile([P, 512], f32, name="qk_ps", tag="qk_ps")
                for dt_i in range(DT):
                    nc.tensor.matmul(
                        ps[:],
                        xvt[:, dt_i, t * P:(t + 1) * P],
                        wv_sb[:, dt_i, :, :],
                        start=(dt_i == 0), stop=(dt_i == DT - 1),
                    )
                nc.vector.tensor_copy(
                    v_all[t][:, :, 0:64], ps[:].rearrange("p (h o) -> p h o", h=HL))
                nc.vector.memset(v_all[t][:, :, 64:65], 1.0)

        # ---- phase B: attention + norm + output projection ----
        with tc.tile_pool(name="et", bufs=6) as et_pool, \
             tc.tile_pool(name="stg", bufs=2) as stg_pool, \
             tc.tile_pool(name="sstg", bufs=2) as sstg_pool, \
             tc.tile_pool(name="ot", bufs=2) as ot_pool, \
             tc.tile_pool(name="lg_psum", bufs=3, space="PSUM") as lg_psum, \
             tc.tile_pool(name="ctx_psum", bufs=1, space="PSUM") as ctx_psum:

            outproj_queue = []

            def outproj_tile(t):
                # out rows [t*128:(t+1)*128] = sum_p ctxn[p]^T @ wo[p]
                ot = ot_pool.tile([P, OUT], f32, name="ot", tag="ot")
                for c in range(OUT // 512):
                    ops = lg_psum.tile([P, 512], f32, name="ops", tag="lg")
                    for p in range(NPAIR):
                        nc.tensor.matmul(
                            ops[:],
                            ctxn[p][:, t * P:(t + 1) * P],
                            wo_sb[:, p, c * 512:(c + 1) * 512],
                            start=(p == 0), stop=(p == NPAIR - 1),
                        )
                    nc.vector.tensor_copy(ot[:, c * 512:(c + 1) * 512], ops[:])
                nc.sync.dma_start(out_ap[t * P:(t + 1) * P, :], ot[:])

            def norm_half(nh):
                # fast reciprocal of the f32 denominators (in place), cast to
                # bf16, PE-broadcast to the pair layout, multiply into ctxn.
                n0 = nh * NH
                nc.vector.reciprocal_approx_fast(
                    sums_all[:, n0:n0 + NH], sums_all[:, n0:n0 + NH])
                nc.vector.tensor_copy(
                    sums_rec[:, n0:n0 + NH], sums_all[:, n0:n0 + NH])
                for p_i in range(NPAIR):
                    bc = lg_psum.tile([P, NH], f32, name="bc", tag="lg")
                    for s in range(2):
                        for c in range(NH // 512):
                            nc.tensor.matmul(
                                bc[s * 64:(s + 1) * 64, c * 512:(c + 1) * 512],
                                hmask_bf[:, 2 * p_i + s, :],
                                sums_rec[:, n0 + c * 512:n0 + (c + 1) * 512],
                                start=True, stop=True,
                            )
                    nc.vector.tensor_mul(
                        ctxn[p_i][:, n0:n0 + NH], ctxn[p_i][:, n0:n0 + NH], bc[:])

            def attention_head(nh, hl):
                n0 = nh * NH
                p_i, s = divmod(hl, 2)
                pr = slice(s * 64, s * 64 + 64)   # partition range of this head
                cps = ctx_psum.tile([P, NH], f32, name="cps", tag="cps")
                ets = {}

                def emit_logits(t):
                    et = et_pool.tile([P, NH], bf16, name="et", tag="et")
                    ets[t] = et
                    lg = lg_psum.tile([P, NH], f32, name="lg", tag="lg")
                    for c in range(NH // 512):
                        nc.tensor.matmul(
                            lg[:, c * 512:(c + 1) * 512],
                            kt[p_i][pr, t * P:(t + 1) * P],
                            qt[p_i][pr, n0 + c * 512:n0 + (c + 1) * 512],
                            start=True, stop=True,
                        )
                    nc.scalar.activation(
                        et[:], lg[:], mybir.ActivationFunctionType.Exp, scale=0.125)

                def emit_ctx(t):
                    et = ets.pop(t)
                    for c in range(NH // 512):
                        nc.tensor.matmul(
                            cps[0:65, c * 512:(c + 1) * 512],
                            v_all[t][:, hl, :],
                            et[:, c * 512:(c + 1) * 512],
                            start=(t == 0), stop=(t == MT - 1),
                        )

                for t in range(MT):
                    emit_logits(t)
                    if t >= PIPE:
                        emit_ctx(t - PIPE)
                    if t == 8 and outproj_queue:
                        outproj_tile(outproj_queue.pop(0))
                    if t == 12 and outproj_queue:
                        outproj_tile(outproj_queue.pop(0))
                for t in range(MT - PIPE, MT):
                    emit_ctx(t)
                # Eviction: ctx rows (bf16) DMA-hop into the pair-stacked
                # ctxn; denominator row (f32) into sums_all[hl].
                stg = stg_pool.tile([64, NH], bf16, name="stg", tag="stg")
                nc.vector.tensor_copy(stg[:], cps[0:64, :])
                nc.sync.dma_start(
                    ctxn[p_i][s * 64:(s + 1) * 64, n0:n0 + NH], stg[:])
                sstg = sstg_pool.tile([1, NH], f32, name="sstg", tag="sstg")
                nc.vector.tensor_copy(sstg[:], cps[64:65, :])
                nc.sync.dma_start(
                    sums_all[hl:hl + 1, n0:n0 + NH], sstg[:])

            for nh in range(N // NH):
                for hl in range(HL):
                    attention_head(nh, hl)
                    if nh == 1 and hl == 0:
                        # half 0 fully evicted; normalize it and queue its
                        # out-projection to drip into half 1's ACT slack.
                        norm_half(0)
                    if nh == 1 and hl == 1:
                        outproj_queue.extend(range(0, 6))
            # tail: reserved out-proj tiles cover the final reciprocal's
            # latency so the PE clock stays promoted
            outproj_tile(6)
            outproj_tile(7)
            norm_half(1)
            for t in range(NT // 2, NT):
                outproj_tile(t)


def build_nc():
    import concourse.bacc as bacc
    import concourse.tile as tile
    from concourse import mybir

    nc = bacc.Bacc("TRN2", target_bir_lowering=False, debug=False)
    f32 = mybir.dt.float32
    ins = {
        "xq": nc.dram_tensor("xq", (N, D), f32, kind="ExternalInput").ap(),
        "xk": nc.dram_tensor("xk", (M, D), f32, kind="ExternalInput").ap(),
        "xv": nc.dram_tensor("xv", (M, D), f32, kind="ExternalInput").ap(),
        "wq": nc.dram_tensor("wq", (HL, D, HS), f32, kind="ExternalInput").ap(),
        "wk": nc.dram_tensor("wk", (HL, D, HS), f32, kind="ExternalInput").ap(),
        "wv": nc.dram_tensor("wv", (HL, D, HS), f32, kind="ExternalInput").ap(),
        "wo": nc.dram_tensor("wo", (HL, HS, OUT), f32, kind="ExternalInput").ap(),
    }
    out_ap = nc.dram_tensor("out", (N, OUT), f32, kind="ExternalOutput").ap()
    with tile.TileContext(nc) as tc:
        build_mha(tc, ins, out_ap)
    nc.compile()
    return nc


def make_in_maps(inputs):
    q = np.ascontiguousarray(np.asarray(inputs["query"], dtype=np.float32))
    k = np.ascontiguousarray(np.asarray(inputs["key"], dtype=np.float32))
    v = np.ascontiguousarray(np.asarray(inputs["value"], dtype=np.float32))
    wq = np.asarray(inputs["query_kernel"], dtype=np.float32)
    wk = np.asarray(inputs["key_kernel"], dtype=np.float32)
    wv = np.asarray(inputs["value_kernel"], dtype=np.float32)
    wo = np.asarray(inputs["projection_kernel"], dtype=np.float32)
    in_maps = []
    for c in range(8):
        b, hg = divmod(c, 2)
        hs = slice(hg * HL, (hg + 1) * HL)
        in_maps.append({
            "xq": q[b], "xk": k[b], "xv": v[b],
            "wq": np.ascontiguousarray(wq[hs]),
            "wk": np.ascontiguousarray(wk[hs]),
            "wv": np.ascontiguousarray(wv[hs]),
            "wo": np.ascontiguousarray(wo[hs]),
        })
    return in_maps


def combine(results, bias):
    out = np.empty((B, N, OUT), dtype=np.float32)
    for b in range(B):
        out[b] = results[2 * b]["out"] + results[2 * b + 1]["out"]
    out += np.asarray(bias, dtype=np.float32)[None, None, :]
    return out


_NC_CACHE = None


def _enable_ldw_opt():
    """No-op (kept for test.py compat). The fp32r baseline forced
    --enable-ldw-opt=true; with bf16 stationaries legalization emits
    standalone InstLdweights which walrus rejects under that flag, and
    the loads pipeline under the matmuls anyway."""
    return


def kernel(**inputs):
    global _NC_CACHE
    from concourse import bass_utils
    _enable_ldw_opt()

    if _NC_CACHE is None:
        _NC_CACHE = build_nc()
    nc = _NC_CACHE
    in_maps = make_in_maps(inputs)
    res = bass_utils.run_bass_kernel_spmd(nc, in_maps, core_ids=list(range(8)))
    return combine(res.results, inputs["projection_bias"])


# revision 64
# speedup vs baseline: 1.0264x; 1.0203x over previous
"""Multi-head attention forward on 8 Trainium2 NeuronCores.

Problem (hardcoded): B=4, N=M=2048, D=1024, H=16, HS=64, OUT=1024, fp32.

Sharding: 8 cores = 4 batches x 2 head-groups of 8 heads. Each core
computes a partial output [2048, 1024] = sum over its 8 heads of
softmax((X_q Wq_h)(X_k Wk_h)^T / 8) (X_v Wv_h) Wo_h.  Host sums the two
head-group partials per batch and adds the projection bias.

Design notes (vs the 749us fp32r baseline):
  * bf16 dataflow everywhere the PE touches data (1 cyc/row, incl.
    transposes which were fp32 = 2 cyc/row); f32 only in PSUM
    accumulators and the softmax denominators. Total rel err ~5e-3
    vs the 2e-2 gate.
  * Phase A is a software pipeline: DMA -> ACT cast to bf16 -> PE
    transpose -> project, with the q projection interleaved into xk's
    transposes (and k-proj into xv's) so the PE instruction stream is
    dense from the first tile on. This keeps the HAM activity monitor
    promoting the PE clock to 2.4 GHz without the baseline's throwaway
    warm-up bursts.
  * Attention loops nh (n-half) outer / head inner. exp runs on the
    ACT engine which is the true bottleneck (33.5M exps/core @ 1.2GHz
    ~ 300us); lg PSUM pool depth 3 lets the PE run up to 3 logit tiles
    ahead so ACT never waits. The output projection of half 0 is
    dripped into half 1's ACT-bound slack.
  * Softmax denominators staged to SBUF in f32; the normalization uses
    reciprocal_approx_fast (~1.3us vs 6.5us for InstReciprocal), and
    two reserved out-projection tiles keep the PE busy across the
    final reciprocal so the clock stays promoted through the tail.
"""

import os
import sys

import numpy as np

for _p in ("/opt/trn_rl_repo",):
    if _p not in sys.path and os.path.isdir(_p):
        sys.path.insert(0, _p)

B, N, M, D = 4, 2048, 2048, 1024
H, HS, OUT = 16, 64, 1024
HL = 8          # heads per core
P = 128
NPAIR = HL // 2  # head pairs per core
DT = D // P      # 8 d-tiles
NT = N // P      # 16 n-tiles
MT = M // P      # 16 m-tiles
NH = 1024       # n-half width for the attention loop
PIPE = 3        # ctx trails logits/exp by this many m-tiles


def build_mha(tc, ins, out_ap):
    import concourse.bass as bass
    from concourse import mybir

    nc = tc.nc
    f32 = mybir.dt.float32
    f32r = mybir.dt.float32r
    bf16 = mybir.dt.bfloat16

    xq, xk, xv = ins["xq"], ins["xk"], ins["xv"]
    wq, wk, wv, wo = ins["wq"], ins["wk"], ins["wv"], ins["wo"]

    import contextlib

    with contextlib.ExitStack() as ctx:
        # ---- constant tiles ----
        const = ctx.enter_context(tc.tile_pool(name="const", bufs=1))
        identity = const.tile([P, P], f32)
        from concourse.masks import make_identity
        make_identity(nc, identity)
        identity_bf = const.tile([P, P], bf16)
        nc.gpsimd.tensor_copy(identity_bf[:], identity[:])
        # head-select masks: hmask[0:HL, h, :] is 1 on partition h, else 0.
        # K=8 lhsT for broadcasting one head's denominator row to 64 partitions.
        hmask_bf = const.tile([HL, HL, 64], bf16)
        nc.gpsimd.memset(hmask_bf[:], 0.0)
        nc.gpsimd.affine_select(
            out=hmask_bf[:],
            in_=hmask_bf[:],
            compare_op=mybir.AluOpType.not_equal,
            fill=1.0,
            base=0,
            # iota = partition - h : zero exactly where partition == h
            pattern=[[-1, HL], [0, 64]],
            channel_multiplier=1,
        )

        # ---- persistent activations ----
        act_pool = ctx.enter_context(tc.tile_pool(name="acts", bufs=1))
        # QT/KT: one [128, 2048] bf16 tile per head pair; partitions 0:64
        # head 2p, 64:128 head 2p+1.
        qt = [act_pool.tile([P, N], bf16, name=f"qt{p}", tag=f"qt{p}") for p in range(NPAIR)]
        kt = [act_pool.tile([P, M], bf16, name=f"kt{p}", tag=f"kt{p}") for p in range(NPAIR)]
        # V: per m-tile [128, 8 heads, 65]; col 0 is ones (softmax denom on
        # PSUM/staging partition 0, where the DVE reciprocal can read it).
        v_all = [act_pool.tile([P, HL, 65], bf16, name=f"v{t}", tag=f"v{t}") for t in range(MT)]
        # un-normalized ctx rows, pair-stacked (even head 0:64, odd 64:128);
        # f32r so one staging copy serves both the ctx rows and the f32
        # denominator row, and the out-proj stationary stays 1 cyc/row
        ctxn = [act_pool.tile([P, N], f32r, name=f"ctxn{p}", tag=f"ctxn{p}")
                for p in range(NPAIR)]
        # per-head bf16 reciprocal denominators, head-stacked on partitions
        # (filled via a partition-0 transient + DMA hop as heads complete)
        sums_recb = act_pool.tile([HL, N], bf16, name="sums_recb", tag="sums_recb")

        # ---- phase A: load, cast, transpose, project (pipelined) ----
        with tc.tile_pool(name="xt", bufs=2) as xt_pool, \
             tc.tile_pool(name="x_stream", bufs=3) as x_stream, \
             tc.tile_pool(name="xb_pool", bufs=3) as xb_pool, \
             tc.tile_pool(name="w_pool", bufs=2) as w_pool, \
             tc.tile_pool(name="w_stream", bufs=2) as w_stream, \
             tc.tile_pool(name="tp_psum", bufs=4, space="PSUM") as tp_psum, \
             tc.tile_pool(name="proj_psum", bufs=3, space="PSUM") as proj_psum:

            def load_w(w_dram, nm):
                # w [8, 1024, 64] -> SBUF [128(d in tile), dt, h, 64] bf16
                w_sb = w_pool.tile([P, DT, HL, HS], bf16, name=nm, tag="w_sb")
                for dt_i in range(DT):
                    st = w_stream.tile([P, HL, HS], f32, name="wst", tag="wst")
                    nc.sync.dma_start(
                        st[:],
                        w_dram[:, dt_i * P:(dt_i + 1) * P, :].rearrange("h p o -> p h o"))
                    nc.scalar.copy(w_sb[:, dt_i, :, :], st[:])
                return w_sb

            def transpose_tile(x_dram, xt_tile, t):
                # one [128, 1024] row-tile: DMA, ACT-cast to bf16, 8 PE
                # transposes into 2 psum groups, evict to xt (DVE).
                x_t = x_stream.tile([P, D], f32, name="x_t", tag="x_t")
                nc.sync.dma_start(x_t[:], x_dram[t * P:(t + 1) * P, :])
                xb = xb_pool.tile([P, D], bf16, name="xb", tag="xb")
                nc.scalar.copy(xb[:], x_t[:])
                for g in range(2):
                    tp = tp_psum.tile([P, 4 * P], bf16, name="tp", tag="tp")
                    for i in range(4):
                        dt_i = 4 * g + i
                        nc.tensor.transpose(
                            tp[:, i * P:(i + 1) * P],
                            xb[:, dt_i * P:(dt_i + 1) * P],
                            identity_bf[:])
                    nc.vector.tensor_copy(
                        xt_tile[:, 4 * g:4 * g + 4, t * P:(t + 1) * P],
                        tp[:].rearrange("p (d n) -> p d n", d=4))

            def qk_proj_chunk(w_sb, xt_tile, dst, p, c):
                ps = proj_psum.tile([P, 512], f32, name="qk_ps", tag="qk_ps")
                for dt_i in range(DT):
                    nc.tensor.matmul(
                        ps[:],
                        w_sb[:, dt_i, 2 * p:2 * p + 2, :],
                        xt_tile[:, dt_i, c * 512:(c + 1) * 512],
                        start=(dt_i == 0), stop=(dt_i == DT - 1),
                    )
                nc.vector.tensor_copy(dst[p][:, c * 512:(c + 1) * 512], ps[:])

            chunks = [(p, c) for p in range(NPAIR) for c in range(N // 512)]

            # q chain: DMA+cast+transpose all 16 tiles (DMA-paced)
            wq_sb = load_w(wq, "wq_sb")
            xqt = xt_pool.tile([P, DT, N], bf16, name="xqt", tag="xT")
            for t in range(NT):
                transpose_tile(xq, xqt, t)
            # k transposes interleaved with q projection (PE stays dense)
            wk_sb = load_w(wk, "wk_sb")
            xkt = xt_pool.tile([P, DT, M], bf16, name="xkt", tag="xT")
            for i in range(NT):
                qk_proj_chunk(wq_sb, xqt, qt, *chunks[i])
                transpose_tile(xk, xkt, i)
            # v transposes interleaved with k projection
            wv_sb = load_w(wv, "wv_sb")
            xvt = xt_pool.tile([P, DT, M], bf16, name="xvt", tag="xT")
            for i in range(NT):
                qk_proj_chunk(wk_sb, xkt, kt, *chunks[i])
                transpose_tile(xv, xvt, i)
            # V projection: v_all[t][:, h, 0:64] = (x_v @ Wv_h)[m-tile t]
            for t in range(MT):
                ps = proj_psum.tile([P, 512], f32, name="qk_ps", tag="qk_ps")
                for dt_i in range(DT):
                    nc.tensor.matmul(
                        ps[:],
                        xvt[:, dt_i, t * P:(t + 1) * P],
                        wv_sb[:, dt_i, :, :],
                        start=(dt_i == 0), stop=(dt_i == DT - 1),
                    )
                nc.vector.tensor_copy(
                    v_all[t][:, :, 1:65], ps[:].rearrange("p (h o) -> p h o", h=HL))
                nc.vector.memset(v_all[t][:, :, 0:1], 1.0)

        # ---- phase B: attention + norm + output projection ----
        with tc.tile_pool(name="et", bufs=6) as et_pool, \
             tc.tile_pool(name="stg", bufs=2) as stg_pool, \
             tc.tile_pool(name="rec", bufs=2) as rec_pool, \
             tc.tile_pool(name="wo_pool", bufs=1) as wo_pool, \
             tc.tile_pool(name="wo_stream", bufs=2) as wo_stream, \
             tc.tile_pool(name="ot", bufs=3) as ot_pool, \
             tc.tile_pool(name="lg_psum", bufs=3, space="PSUM") as lg_psum, \
             tc.tile_pool(name="ctx_psum", bufs=1, space="PSUM") as ctx_psum:

            # wo [8, 64, 1024] -> SBUF [128(s*64+o), pair, 1024] f32r
            # (pairs the f32r ctxn stationary; DVE casts, ACT is busy)
            wo_sb = wo_pool.tile([P, NPAIR, OUT], f32r, name="wo_sb", tag="wo_sb")
            for s in range(2):
                for c4 in range(4):
                    wst = wo_stream.tile([64, NPAIR, 256], f32, name="wo_st", tag="wo_st")
                    nc.sync.dma_start(
                        wst[:],
                        wo[s::2, :, c4 * 256:(c4 + 1) * 256].rearrange("pp o d -> o pp d"))
                    nc.vector.tensor_copy(
                        wo_sb[s * 64:(s + 1) * 64, :, c4 * 256:(c4 + 1) * 256], wst[:])

            outproj_queue = []

            def outproj_tile(t):
                # out rows [t*128:(t+1)*128] = sum_p ctxn[p]^T @ wo[p]
                ot = ot_pool.tile([P, OUT], f32, name="ot", tag="ot")
                for c in range(OUT // 512):
                    ops = lg_psum.tile([P, 512], f32, name="ops", tag="lg")
                    for p in range(NPAIR):
                        nc.tensor.matmul(
                            ops[:],
                            ctxn[p][:, t * P:(t + 1) * P],
                            wo_sb[:, p, c * 512:(c + 1) * 512],
                            start=(p == 0), stop=(p == NPAIR - 1),
                        )
                    nc.vector.tensor_copy(ot[:, c * 512:(c + 1) * 512], ops[:])
                nc.sync.dma_start(out_ap[t * P:(t + 1) * P, :], ot[:])

            def norm_half(nh):
                # PE-broadcast the per-head reciprocals to the pair layout,
                # multiply into ctxn.
                n0 = nh * NH
                for p_i in range(NPAIR):
                    bc = lg_psum.tile([P, NH], f32, name="bc", tag="lg")
                    for s in range(2):
                        for c in range(NH // 512):
                            nc.tensor.matmul(
                                bc[s * 64:(s + 1) * 64, c * 512:(c + 1) * 512],
                                hmask_bf[:, 2 * p_i + s, :],
                                sums_recb[:, n0 + c * 512:n0 + (c + 1) * 512],
                                start=True, stop=True,
                            )
                    nc.vector.tensor_mul(
                        ctxn[p_i][:, n0:n0 + NH], ctxn[p_i][:, n0:n0 + NH], bc[:])

            def attention_head(nh, hl):
                n0 = nh * NH
                p_i, s = divmod(hl, 2)
                pr = slice(s * 64, s * 64 + 64)   # partition range of this head
                cps = ctx_psum.tile([P, NH], f32, name="cps", tag="cps")
                ets = {}

                def emit_logits(t):
                    et = et_pool.tile([P, NH], bf16, name="et", tag="et")
                    ets[t] = et
                    lg = lg_psum.tile([P, NH], f32, name="lg", tag="lg")
                    for c in range(NH // 512):
                        nc.tensor.matmul(
                            lg[:, c * 512:(c + 1) * 512],
                            kt[p_i][pr, t * P:(t + 1) * P],
                            qt[p_i][pr, n0 + c * 512:n0 + (c + 1) * 512],
                            start=True, stop=True,
                        )
                    nc.scalar.activation(
                        et[:], lg[:], mybir.ActivationFunctionType.Exp, scale=0.125)

                def emit_ctx(t):
                    et = ets.pop(t)
                    for c in range(NH // 512):
                        nc.tensor.matmul(
                            cps[0:65, c * 512:(c + 1) * 512],
                            v_all[t][:, hl, :],
                            et[:, c * 512:(c + 1) * 512],
                            start=(t == 0), stop=(t == MT - 1),
                        )

                for t in range(MT):
                    emit_logits(t)
                    if t >= PIPE:
                        emit_ctx(t - PIPE)
                    if t == 8 and outproj_queue:
                        outproj_tile(outproj_queue.pop(0))
                    if t == 12 and outproj_queue:
                        outproj_tile(outproj_queue.pop(0))
                for t in range(MT - PIPE, MT):
                    emit_ctx(t)
                # Eviction: ONE staging copy (so cps frees fast), then DMA
                # the ctx rows (1:65) into the pair-stacked ctxn; the
                # denominator reciprocal is computed off staging row 0.
                stg = stg_pool.tile([65, NH], f32r, name="stg", tag="stg")
                nc.vector.tensor_copy(stg[:], cps[0:65, :])
                nc.sync.dma_start(
                    ctxn[p_i][s * 64:(s + 1) * 64, n0:n0 + NH], stg[1:65, :])
                rrow = rec_pool.tile([1, NH], f32, name="rrow", tag="rrow")
                nc.vector.reciprocal_approx_fast(
                    rrow[:], stg[0:1, :].bitcast(f32))
                rrowb = rec_pool.tile([1, NH], bf16, name="rrowb", tag="rrowb")
                nc.vector.tensor_copy(rrowb[:], rrow[:])
                nc.sync.dma_start(
                    sums_recb[hl:hl + 1, n0:n0 + NH], rrowb[:])

            for nh in range(N // NH):
                for hl in range(HL):
                    attention_head(nh, hl)
                    if nh == 1 and hl == 0:
                        # half 0 fully evicted; normalize it and queue its
                        # out-projection to drip into half 1's ACT slack.
                        norm_half(0)
                    if nh == 1 and hl == 1:
                        outproj_queue.extend(range(0, 6))
            # tail: reserved out-proj tiles cover the final reciprocal's
            # latency so the PE clock stays promoted
            outproj_tile(6)
            outproj_tile(7)
            norm_half(1)
            for t in range(NT // 2, NT):
                outproj_tile(t)


def build_nc():
    import concourse.bacc as bacc
    import concourse.tile as tile
    from concourse import mybir

    nc = bacc.Bacc("TRN2", target_bir_lowering=False, debug=False)
    f32 = mybir.dt.float32
    ins = {
        "xq": nc.dram_tensor("xq", (N, D), f32, kind="ExternalInput").ap(),
        "xk": nc.dram_tensor("xk", (M, D), f32, kind="ExternalInput").ap(),
        "xv": nc.dram_tensor("xv", (M, D), f32, kind="ExternalInput").ap(),
        "wq": nc.dram_tensor("wq", (HL, D, HS), f32, kind="ExternalInput").ap(),
        "wk": nc.dram_tensor("wk", (HL, D, HS), f32, kind="ExternalInput").ap(),
        "wv": nc.dram_tensor("wv", (HL, D, HS), f32, kind="ExternalInput").ap(),
        "wo": nc.dram_tensor("wo", (HL, HS, OUT), f32, kind="ExternalInput").ap(),
    }
    out_ap = nc.dram_tensor("out", (N, OUT), f32, kind="ExternalOutput").ap()
    with tile.TileContext(nc) as tc:
        build_mha(tc, ins, out_ap)
    nc.compile()
    return nc


def make_in_maps(inputs):
    q = np.ascontiguousarray(np.asarray(inputs["query"], dtype=np.float32))
    k = np.ascontiguousarray(np.asarray(inputs["key"], dtype=np.float32))
    v = np.ascontiguousarray(np.asarray(inputs["value"], dtype=np.float32))
    wq = np.asarray(inputs["query_kernel"], dtype=np.float32)
    wk = np.asarray(inputs["key_kernel"], dtype=np.float32)
    wv = np.asarray(inputs["value_kernel"], dtype=np.float32)
    wo = np.asarray(inputs["projection_kernel"], dtype=np.float32)
    in_maps = []
    for c in range(8):
        b, hg = divmod(c, 2)
        hs = slice(hg * HL, (hg + 1) * HL)
        in_maps.append({
            "xq": q[b], "xk": k[b], "xv": v[b],
            "wq": np.ascontiguousarray(wq[hs]),
            "wk": np.ascontiguousarray(wk[hs]),
            "wv": np.ascontiguousarray(wv[hs]),
            "wo": np.ascontiguousarray(wo[hs]),
        })
    return in_maps


def combine(results, bias):
    out = np.empty((B, N, OUT), dtype=np.float32)
    for b in range(B):
        out[b] = results[2 * b]["out"] + results[2 * b + 1]["out"]
    out += np.asarray(bias, dtype=np.float32)[None, None, :]
    return out


_NC_CACHE = None


def _enable_ldw_opt():
    """No-op (kept for test.py compat). The fp32r baseline forced
    --enable-ldw-opt=true; with bf16 stationaries legalization emits
    standalone InstLdweights which walrus rejects under that flag, and
    the loads pipeline under the matmuls anyway."""
    return


def kernel(**inputs):
    global _NC_CACHE
    from concourse import bass_utils
    _enable_ldw_opt()

    if _NC_CACHE is None:
        _NC_CACHE = build_nc()
    nc = _NC_CACHE
    in_maps = make_in_maps(inputs)
    res = bass_utils.run_bass_kernel_spmd(nc, in_maps, core_ids=list(range(8)))
    return combine(res.results, inputs["projection_bias"])


# revision 66
# speedup vs baseline: 1.0395x; 1.0127x over previous
"""Multi-head attention forward on 8 Trainium2 NeuronCores.

Problem (hardcoded): B=4, N=M=2048, D=1024, H=16, HS=64, OUT=1024, fp32.

Sharding: 8 cores = 4 batches x 2 head-groups of 8 heads. Each core
computes a partial output [2048, 1024] = sum over its 8 heads of
softmax((X_q Wq_h)(X_k Wk_h)^T / 8) (X_v Wv_h) Wo_h.  Host sums the two
head-group partials per batch and adds the projection bias.

Design notes (vs the 749us fp32r baseline):
  * bf16 dataflow everywhere the PE touches data (1 cyc/row, incl.
    transposes which were fp32 = 2 cyc/row); f32 only in PSUM
    accumulators and the softmax denominators. Total rel err ~5e-3
    vs the 2e-2 gate.
  * Phase A is a software pipeline: DMA -> ACT cast to bf16 -> PE
    transpose -> project, with the q projection interleaved into xk's
    transposes (and k-proj into xv's) so the PE instruction stream is
    dense from the first tile on. This keeps the HAM activity monitor
    promoting the PE clock to 2.4 GHz without the baseline's throwaway
    warm-up bursts.
  * Attention loops nh (n-half) outer / head inner. exp runs on the
    ACT engine which is the true bottleneck (33.5M exps/core @ 1.2GHz
    ~ 300us); lg PSUM pool depth 3 lets the PE run up to 3 logit tiles
    ahead so ACT never waits. The output projection of half 0 is
    dripped into half 1's ACT-bound slack.
  * Softmax denominators staged to SBUF in f32; the normalization uses
    reciprocal_approx_fast (~1.3us vs 6.5us for InstReciprocal), and
    two reserved out-projection tiles keep the PE busy across the
    final reciprocal so the clock stays promoted through the tail.
"""

import os
import sys

import numpy as np

for _p in ("/opt/trn_rl_repo",):
    if _p not in sys.path and os.path.isdir(_p):
        sys.path.insert(0, _p)

B, N, M, D = 4, 2048, 2048, 1024
H, HS, OUT = 16, 64, 1024
HL = 8          # heads per core
P = 128
NPAIR = HL // 2  # head pairs per core
DT = D // P      # 8 d-tiles
NT = N // P      # 16 n-tiles
MT = M // P      # 16 m-tiles
NH = 1024       # n-half width for the attention loop
PIPE = 3        # ctx trails logits/exp by this many m-tiles


def build_mha(tc, ins, out_ap):
    import concourse.bass as bass
    from concourse import mybir

    nc = tc.nc
    f32 = mybir.dt.float32
    f32r = mybir.dt.float32r
    bf16 = mybir.dt.bfloat16

    xq, xk, xv = ins["xq"], ins["xk"], ins["xv"]
    wq, wk, wv, wo = ins["wq"], ins["wk"], ins["wv"], ins["wo"]

    import contextlib

    with contextlib.ExitStack() as ctx:
        # ---- constant tiles ----
        const = ctx.enter_context(tc.tile_pool(name="const", bufs=1))
        identity = const.tile([P, P], f32)
        from concourse.masks import make_identity
        make_identity(nc, identity)
        identity_bf = const.tile([P, P], bf16)
        nc.gpsimd.tensor_copy(identity_bf[:], identity[:])
        # head-select masks: hmask[0:HL, h, :] is 1 on partition h, else 0.
        # K=8 lhsT for broadcasting one head's denominator row to 64 partitions.
        hmask_bf = const.tile([HL, HL, 64], bf16)
        nc.gpsimd.memset(hmask_bf[:], 0.0)
        nc.gpsimd.affine_select(
            out=hmask_bf[:],
            in_=hmask_bf[:],
            compare_op=mybir.AluOpType.not_equal,
            fill=1.0,
            base=0,
            # iota = partition - h : zero exactly where partition == h
            pattern=[[-1, HL], [0, 64]],
            channel_multiplier=1,
        )

        # ---- persistent activations ----
        act_pool = ctx.enter_context(tc.tile_pool(name="acts", bufs=1))
        # QT/KT: one [128, 2048] bf16 tile per head pair; partitions 0:64
        # head 2p, 64:128 head 2p+1.
        qt = [act_pool.tile([P, N], bf16, name=f"qt{p}", tag=f"qt{p}") for p in range(NPAIR)]
        kt = [act_pool.tile([P, M], bf16, name=f"kt{p}", tag=f"kt{p}") for p in range(NPAIR)]
        # V: per m-tile [128, 8 heads, 65]; col 0 is ones (softmax denom on
        # PSUM/staging partition 0, where the DVE reciprocal can read it).
        v_all = [act_pool.tile([P, HL, 65], bf16, name=f"v{t}", tag=f"v{t}") for t in range(MT)]
        # un-normalized ctx rows, pair-stacked (even head 0:64, odd 64:128);
        # f32r so one staging copy serves both the ctx rows and the f32
        # denominator row, and the out-proj stationary stays 1 cyc/row
        ctxn = [act_pool.tile([P, N], f32r, name=f"ctxn{p}", tag=f"ctxn{p}")
                for p in range(NPAIR)]
        # per-head bf16 reciprocal denominators, head-stacked on partitions
        # (filled via a partition-0 transient + DMA hop as heads complete)
        sums_recb = act_pool.tile([HL, N], bf16, name="sums_recb", tag="sums_recb")

        # ---- phase A: load, cast, transpose, project (pipelined) ----
        with tc.tile_pool(name="xt", bufs=2) as xt_pool, \
             tc.tile_pool(name="x_stream", bufs=3) as x_stream, \
             tc.tile_pool(name="xb_pool", bufs=3) as xb_pool, \
             tc.tile_pool(name="w_pool", bufs=2) as w_pool, \
             tc.tile_pool(name="w_stream", bufs=2) as w_stream, \
             tc.tile_pool(name="tp_psum", bufs=4, space="PSUM") as tp_psum, \
             tc.tile_pool(name="proj_psum", bufs=3, space="PSUM") as proj_psum:

            def load_w(w_dram, nm):
                # w [8, 1024, 64] -> SBUF [128(d in tile), dt, h, 64] bf16
                w_sb = w_pool.tile([P, DT, HL, HS], bf16, name=nm, tag="w_sb")
                for dt_i in range(DT):
                    st = w_stream.tile([P, HL, HS], f32, name="wst", tag="wst")
                    nc.sync.dma_start(
                        st[:],
                        w_dram[:, dt_i * P:(dt_i + 1) * P, :].rearrange("h p o -> p h o"))
                    nc.scalar.copy(w_sb[:, dt_i, :, :], st[:])
                return w_sb

            def transpose_tile(x_dram, xt_tile, t):
                # one [128, 1024] row-tile: DMA, ACT-cast to bf16, 8 PE
                # transposes into 2 psum groups, evict to xt (DVE).
                x_t = x_stream.tile([P, D], f32, name="x_t", tag="x_t")
                nc.sync.dma_start(x_t[:], x_dram[t * P:(t + 1) * P, :])
                xb = xb_pool.tile([P, D], bf16, name="xb", tag="xb")
                nc.scalar.copy(xb[:], x_t[:])
                for g in range(2):
                    tp = tp_psum.tile([P, 4 * P], bf16, name="tp", tag="tp")
                    for i in range(4):
                        dt_i = 4 * g + i
                        nc.tensor.transpose(
                            tp[:, i * P:(i + 1) * P],
                            xb[:, dt_i * P:(dt_i + 1) * P],
                            identity_bf[:])
                    nc.vector.tensor_copy(
                        xt_tile[:, 4 * g:4 * g + 4, t * P:(t + 1) * P],
                        tp[:].rearrange("p (d n) -> p d n", d=4))

            def qk_proj_chunk(w_sb, xt_tile, dst, p, c):
                ps = proj_psum.tile([P, 512], f32, name="qk_ps", tag="qk_ps")
                for dt_i in range(DT):
                    nc.tensor.matmul(
                        ps[:],
                        w_sb[:, dt_i, 2 * p:2 * p + 2, :],
                        xt_tile[:, dt_i, c * 512:(c + 1) * 512],
                        start=(dt_i == 0), stop=(dt_i == DT - 1),
                    )
                nc.vector.tensor_copy(dst[p][:, c * 512:(c + 1) * 512], ps[:])

            chunks = [(p, c) for p in range(NPAIR) for c in range(N // 512)]

            # q chain: DMA+cast+transpose all 16 tiles (DMA-paced)
            wq_sb = load_w(wq, "wq_sb")
            xqt = xt_pool.tile([P, DT, N], bf16, name="xqt", tag="xT")
            for t in range(NT):
                transpose_tile(xq, xqt, t)
            # k transposes interleaved with q projection (PE stays dense)
            wk_sb = load_w(wk, "wk_sb")
            xkt = xt_pool.tile([P, DT, M], bf16, name="xkt", tag="xT")
            for i in range(NT):
                qk_proj_chunk(wq_sb, xqt, qt, *chunks[i])
                transpose_tile(xk, xkt, i)
            # v transposes interleaved with k projection
            wv_sb = load_w(wv, "wv_sb")
            xvt = xt_pool.tile([P, DT, M], bf16, name="xvt", tag="xT")
            for i in range(NT):
                qk_proj_chunk(wk_sb, xkt, kt, *chunks[i])
                transpose_tile(xv, xvt, i)
            # V projection: v_all[t][:, h, 0:64] = (x_v @ Wv_h)[m-tile t]
            for t in range(MT):
                ps = proj_psum.tile([P, 512], f32, name="qk_ps", tag="qk_ps")
                for dt_i in range(DT):
                    nc.tensor.matmul(
                        ps[:],
                        xvt[:, dt_i, t * P:(t + 1) * P],
                        wv_sb[:, dt_i, :, :],
                        start=(dt_i == 0), stop=(dt_i == DT - 1),
                    )
                nc.vector.tensor_copy(
                    v_all[t][:, :, 1:65], ps[:].rearrange("p (h o) -> p h o", h=HL))
                nc.vector.memset(v_all[t][:, :, 0:1], 1.0)

        # ---- phase B: attention + norm + output projection ----
        with tc.tile_pool(name="et", bufs=6) as et_pool, \
             tc.tile_pool(name="stg", bufs=2) as stg_pool, \
             tc.tile_pool(name="rec", bufs=2) as rec_pool, \
             tc.tile_pool(name="wo_pool", bufs=1) as wo_pool, \
             tc.tile_pool(name="wo_stream", bufs=2) as wo_stream, \
             tc.tile_pool(name="ot", bufs=3) as ot_pool, \
             tc.tile_pool(name="lg_psum", bufs=3, space="PSUM") as lg_psum, \
             tc.tile_pool(name="ctx_psum", bufs=1, space="PSUM") as ctx_psum:

            # wo [8, 64, 1024] -> SBUF [128(s*64+o), pair, 1024] f32r
            # (pairs the f32r ctxn stationary; DVE casts, ACT is busy)
            wo_sb = wo_pool.tile([P, NPAIR, OUT], f32r, name="wo_sb", tag="wo_sb")
            for s in range(2):
                for c4 in range(4):
                    wst = wo_stream.tile([64, NPAIR, 256], f32, name="wo_st", tag="wo_st")
                    nc.sync.dma_start(
                        wst[:],
                        wo[s::2, :, c4 * 256:(c4 + 1) * 256].rearrange("pp o d -> o pp d"))
                    nc.vector.tensor_copy(
                        wo_sb[s * 64:(s + 1) * 64, :, c4 * 256:(c4 + 1) * 256], wst[:])

            outproj_queue = []

            def outproj_tile(t):
                # out rows [t*128:(t+1)*128] = sum_p ctxn[p]^T @ wo[p]
                ot = ot_pool.tile([P, OUT], f32, name="ot", tag="ot")
                for c in range(OUT // 512):
                    ops = lg_psum.tile([P, 512], f32, name="ops", tag="lg")
                    for p in range(NPAIR):
                        nc.tensor.matmul(
                            ops[:],
                            ctxn[p][:, t * P:(t + 1) * P],
                            wo_sb[:, p, c * 512:(c + 1) * 512],
                            start=(p == 0), stop=(p == NPAIR - 1),
                        )
                    nc.vector.tensor_copy(ot[:, c * 512:(c + 1) * 512], ops[:])
                nc.sync.dma_start(out_ap[t * P:(t + 1) * P, :], ot[:])

            def norm_half(nh):
                # PE-broadcast the per-head reciprocals to the pair layout,
                # multiply into ctxn.
                n0 = nh * NH
                for p_i in range(NPAIR):
                    bc = lg_psum.tile([P, NH], f32, name="bc", tag="lg")
                    for s in range(2):
                        for c in range(NH // 512):
                            nc.tensor.matmul(
                                bc[s * 64:(s + 1) * 64, c * 512:(c + 1) * 512],
                                hmask_bf[:, 2 * p_i + s, :],
                                sums_recb[:, n0 + c * 512:n0 + (c + 1) * 512],
                                start=True, stop=True,
                            )
                    nc.vector.tensor_mul(
                        ctxn[p_i][:, n0:n0 + NH], ctxn[p_i][:, n0:n0 + NH], bc[:])

            def attention_head(nh, hl):
                n0 = nh * NH
                p_i, s = divmod(hl, 2)
                pr = slice(s * 64, s * 64 + 64)   # partition range of this head
                if not outproj_queue:
                    # HAM re-promotion burst: one gapless 10-matmul
                    # accumulation chain (~4096+ cycles). If the PE clock
                    # demoted (ACT-paced loops never re-promote on their
                    # own), this restores 2.4GHz; if promoted, it costs
                    # ~2.1us absorbed by the ACT-bound slack. Heads with
                    # queued fillers get their chains from those instead.
                    warm = lg_psum.tile([P, NH], f32, name="warm", tag="lg")
                    for w in range(10):
                        nc.tensor.matmul(
                            warm[:, 0:512],
                            kt[p_i][pr, (w % MT) * P:((w % MT) + 1) * P],
                            qt[p_i][pr, n0:n0 + 512],
                            start=(w == 0), stop=(w == 9),
                            skip_group_check=True,
                        )
                cps = ctx_psum.tile([P, NH], f32, name="cps", tag="cps")
                ets = {}

                def emit_logits(t):
                    et = et_pool.tile([P, NH], bf16, name="et", tag="et")
                    ets[t] = et
                    lg = lg_psum.tile([P, NH], f32, name="lg", tag="lg")
                    for c in range(NH // 512):
                        nc.tensor.matmul(
                            lg[:, c * 512:(c + 1) * 512],
                            kt[p_i][pr, t * P:(t + 1) * P],
                            qt[p_i][pr, n0 + c * 512:n0 + (c + 1) * 512],
                            start=True, stop=True,
                        )
                    nc.scalar.activation(
                        et[:], lg[:], mybir.ActivationFunctionType.Exp, scale=0.125)

                def emit_ctx(t):
                    et = ets.pop(t)
                    for c in range(NH // 512):
                        nc.tensor.matmul(
                            cps[0:65, c * 512:(c + 1) * 512],
                            v_all[t][:, hl, :],
                            et[:, c * 512:(c + 1) * 512],
                            start=(t == 0), stop=(t == MT - 1),
                        )

                for t in range(MT):
                    emit_logits(t)
                    if t >= PIPE:
                        emit_ctx(t - PIPE)
                    if t == 8 and outproj_queue:
                        outproj_tile(outproj_queue.pop(0))
                    if t == 12 and outproj_queue:
                        outproj_tile(outproj_queue.pop(0))
                for t in range(MT - PIPE, MT):
                    emit_ctx(t)
                # Eviction: ONE staging copy (so cps frees fast), then DMA
                # the ctx rows (1:65) into the pair-stacked ctxn; the
                # denominator reciprocal is computed off staging row 0.
                stg = stg_pool.tile([65, NH], f32r, name="stg", tag="stg")
                nc.vector.tensor_copy(stg[:], cps[0:65, :])
                nc.sync.dma_start(
                    ctxn[p_i][s * 64:(s + 1) * 64, n0:n0 + NH], stg[1:65, :])
                rrow = rec_pool.tile([1, NH], f32, name="rrow", tag="rrow")
                nc.vector.reciprocal_approx_fast(
                    rrow[:], stg[0:1, :].bitcast(f32))
                rrowb = rec_pool.tile([1, NH], bf16, name="rrowb", tag="rrowb")
                nc.vector.tensor_copy(rrowb[:], rrow[:])
                nc.sync.dma_start(
                    sums_recb[hl:hl + 1, n0:n0 + NH], rrowb[:])

            for nh in range(N // NH):
                for hl in range(HL):
                    attention_head(nh, hl)
                    if nh == 1 and hl == 0:
                        # half 0 fully evicted; normalize it and queue its
                        # out-projection to drip into half 1's ACT slack.
                        norm_half(0)
                    if nh == 1 and hl == 1:
                        outproj_queue.extend(range(0, 6))
            # tail: reserved out-proj tiles cover the final reciprocal's
            # latency so the PE clock stays promoted
            outproj_tile(6)
            outproj_tile(7)
            norm_half(1)
            for t in range(NT // 2, NT):
                outproj_tile(t)


def build_nc():
    import concourse.bacc as bacc
    import concourse.tile as tile
    from concourse import mybir

    nc = bacc.Bacc("TRN2", target_bir_lowering=False, debug=False)
    f32 = mybir.dt.float32
    ins = {
        "xq": nc.dram_tensor("xq", (N, D), f32, kind="ExternalInput").ap(),
        "xk": nc.dram_tensor("xk", (M, D), f32, kind="ExternalInput").ap(),
        "xv": nc.dram_tensor("xv", (M, D), f32, kind="ExternalInput").ap(),
        "wq": nc.dram_tensor("wq", (HL, D, HS), f32, kind="ExternalInput").ap(),
        "wk": nc.dram_tensor("wk", (HL, D, HS), f32, kind="ExternalInput").ap(),
        "wv": nc.dram_tensor("wv", (HL, D, HS), f32, kind="ExternalInput").ap(),
        "wo": nc.dram_tensor("wo", (HL, HS, OUT), f32, kind="ExternalInput").ap(),
    }
    out_ap = nc.dram_tensor("out", (N, OUT), f32, kind="ExternalOutput").ap()
    with tile.TileContext(nc) as tc:
        build_mha(tc, ins, out_ap)
    nc.compile()
    return nc


def make_in_maps(inputs):
    q = np.ascontiguousarray(np.asarray(inputs["query"], dtype=np.float32))
    k = np.ascontiguousarray(np.asarray(inputs["key"], dtype=np.float32))
    v = np.ascontiguousarray(np.asarray(inputs["value"], dtype=np.float32))
    wq = np.asarray(inputs["query_kernel"], dtype=np.float32)
    wk = np.asarray(inputs["key_kernel"], dtype=np.float32)
    wv = np.asarray(inputs["value_kernel"], dtype=np.float32)
    wo = np.asarray(inputs["projection_kernel"], dtype=np.float32)
    in_maps = []
    for c in range(8):
        b, hg = divmod(c, 2)
        hs = slice(hg * HL, (hg + 1) * HL)
        in_maps.append({
            "xq": q[b], "xk": k[b], "xv": v[b],
            "wq": np.ascontiguousarray(wq[hs]),
            "wk": np.ascontiguousarray(wk[hs]),
            "wv": np.ascontiguousarray(wv[hs]),
            "wo": np.ascontiguousarray(wo[hs]),
        })
    return in_maps


def combine(results, bias):
    out = np.empty((B, N, OUT), dtype=np.float32)
    for b in range(B):
        out[b] = results[2 * b]["out"] + results[2 * b + 1]["out"]
    out += np.asarray(bias, dtype=np.float32)[None, None, :]
    return out


_NC_CACHE = None


def _enable_ldw_opt():
    """No-op (kept for test.py compat). The fp32r baseline forced
    --enable-ldw-opt=true; with bf16 stationaries legalization emits
    standalone InstLdweights which walrus rejects under that flag, and
    the loads pipeline under the matmuls anyway."""
    return


def kernel(**inputs):
    global _NC_CACHE
    from concourse import bass_utils
    _enable_ldw_opt()

    if _NC_CACHE is None:
        _NC_CACHE = build_nc()
    nc = _NC_CACHE
    in_maps = make_in_maps(inputs)
    res = bass_utils.run_bass_kernel_spmd(nc, in_maps, core_ids=list(range(8)))
    return combine(res.results, inputs["projection_bias"])
